# revision 9
# baseline (speedup 1.0000x reference)
"""Complex batch-norm Trainium2 kernel (nn_ComplexBatchNormal).

Full inputs: x_real/x_imag [16, 32, 256, 256] f32, params [32, 256, 256] f32.
Output: complex64 [16, 32, 256, 256].

Sharding: channels C=32 split across 8 cores (4 channels each) -> fully local
batch statistics per core, no collectives.

Per-core algorithm (positions N = 4*256*256 = 262144, batch B = 16, 4 tiles
of [128, 512] positions):
  pass 1: load x f32, cast to fp16 (ACT for xr, DVE for xi), squares on ACT,
          product on DVE; 5 stats accumulated over B via TensorE identity
          matmuls into 5 PSUM banks.
  coef:   per tile-PAIR (FD=1024 ops), fp16 throughout: analytic inverse-sqrt
          of the 2x2 covariance folded with gamma/beta/mu into 6 fp16
          coefficients a1,a2,a0,b1,b2,b0.
  pass 2: DVE muls U=a1*xr, V=a2*xi (fp16 2x, quarter-batch granularity);
          U+V+bias summed on TensorE into PSUM (3 streams/comp/sample);
          per-sample PSUM->SBUF fp16 copyouts split DVE/ACT; outputs are two
          fp16 DRAM tensors (re, im) upcast + combined on host.
"""

import sys

if "/opt/trn_rl_repo" not in sys.path:
    sys.path.insert(0, "/opt/trn_rl_repo")

from contextlib import ExitStack

import numpy as np

import concourse.bacc as bacc
import concourse.bass as bass
import concourse.tile as tile
from concourse import masks, mybir
from concourse.bass_utils import run_bass_kernel_spmd

P = 128          # SBUF partitions
F = 512          # free-dim positions per tile (= one PSUM bank of f32)
NB = 16          # batch size
G = 4            # batch-samples per load group
QB = 2           # batch-samples per pass-2 mul/out group
EPS = 1e-5
N_CORES = 8
C_FULL = 32
C_LOC = C_FULL // N_CORES  # 4 channels per core
HW = 256 * 256
NPOS_FULL = C_LOC * HW     # 262144 positions per core

f32 = mybir.dt.float32
f16 = mybir.dt.float16

ACT = mybir.ActivationFunctionType


def bcast_free(ap: bass.AP, n: int) -> bass.AP:
    """View [P, F] as [P, n, F] with the middle dim broadcast (step 0)."""
    return bass.AP(tensor=ap.tensor, offset=ap.offset, ap=[ap.ap[0], [0, n], ap.ap[1]])


def _emit(nc: bacc.Bacc, ctx: ExitStack, tc: "tile.TileContext", npos: int,
          dve_copyout_share: int = 3):
    NT = npos // (P * F)
    assert NT * P * F == npos and NT % 2 == 0

    xr_d = nc.dram_tensor("xr", [NB, npos], f32, kind="ExternalInput")
    xi_d = nc.dram_tensor("xi", [NB, npos], f32, kind="ExternalInput")
    grr_d = nc.dram_tensor("grr", [npos], f32, kind="ExternalInput")
    gri_d = nc.dram_tensor("gri", [npos], f32, kind="ExternalInput")
    gii_d = nc.dram_tensor("gii", [npos], f32, kind="ExternalInput")
    bet_d = nc.dram_tensor("bet", [npos], f32, kind="ExternalInput")
    or_d = nc.dram_tensor("outr", [NB, npos], f16, kind="ExternalOutput")
    oi_d = nc.dram_tensor("outi", [NB, npos], f16, kind="ExternalOutput")

    xr_gv = xr_d.ap().rearrange("(g q) (t p f) -> g t p q f", q=G, p=P, f=F)
    xi_gv = xi_d.ap().rearrange("(g q) (t p f) -> g t p q f", q=G, p=P, f=F)
    grr_v = grr_d.ap().rearrange("(t p f) -> t p f", p=P, f=F)
    gri_v = gri_d.ap().rearrange("(t p f) -> t p f", p=P, f=F)
    gii_v = gii_d.ap().rearrange("(t p f) -> t p f", p=P, f=F)
    bet_v = bet_d.ap().rearrange("(t p f) -> t p f", p=P, f=F)
    or_v = or_d.ap().rearrange("(q b) (t p f) -> q t p b f", b=QB, p=P, f=F)
    oi_v = oi_d.ap().rearrange("(q b) (t p f) -> q t p b f", b=QB, p=P, f=F)

    singles = ctx.enter_context(tc.tile_pool(name="singles", bufs=1))
    xpool = ctx.enter_context(tc.tile_pool(name="x", bufs=2))      # f32 staging
    xbpool = ctx.enter_context(tc.tile_pool(name="xb", bufs=2))    # fp16 resident
    sqpool = ctx.enter_context(tc.tile_pool(name="sq", bufs=1))
    gpool = ctx.enter_context(tc.tile_pool(name="g", bufs=1))
    epool = ctx.enter_context(tc.tile_pool(name="ex", bufs=1))     # stat extracts (pairs)
    cpool = ctx.enter_context(tc.tile_pool(name="coef", bufs=2))   # coef scratch (pairs)
    wpool = ctx.enter_context(tc.tile_pool(name="w", bufs=1))      # U/V tiles
    opool = ctx.enter_context(tc.tile_pool(name="o", bufs=2))      # fp16 out tiles
    psum = ctx.enter_context(tc.tile_pool(name="ps", bufs=1, space="PSUM"))

    ident = singles.tile([P, P], f32)
    masks.make_identity(nc, ident[:])
    identb = singles.tile([P, P], f16)
    nc.scalar.copy(identb[:], ident[:])

    inv16 = 1.0 / NB

    NPAIR = NT // 2

    # per-pair persistent handles
    ex_tiles = {}
    coef_tiles = {}

    def pass1(t):
        """Load + cast + squares + product + stat matmuls for tile t."""
        XB = xbpool.tile([P, NB, F], f16, tag="XB", name=f"XB{t}")
        XIB = xbpool.tile([P, NB, F], f16, tag="XIB", name=f"XIB{t}")
        for g in range(NB // G):
            xg = xpool.tile([P, G, F], f32, tag="xr", name=f"xr{t}_{g}")
            nc.sync.dma_start(xg[:], xr_gv[g, t])
            yg = xpool.tile([P, G, F], f32, tag="xi", name=f"xi{t}_{g}")
            nc.sync.dma_start(yg[:], xi_gv[g, t])

            XBg = XB[:, g * G:(g + 1) * G, :]
            XIBg = XIB[:, g * G:(g + 1) * G, :]
            nc.scalar.copy(XBg, xg[:])            # ACT cast
            nc.vector.tensor_copy(XIBg, yg[:])    # DVE cast (2x-2p)

            sq_r = sqpool.tile([P, G, F], f16, tag="sqr", name=f"sqr{t}_{g}")
            sq_i = sqpool.tile([P, G, F], f16, tag="sqi", name=f"sqi{t}_{g}")
            p_g = sqpool.tile([P, G, F], f16, tag="pg", name=f"pg{t}_{g}")
            nc.scalar.square(sq_r[:], xg[:])      # ACT
            nc.scalar.square(sq_i[:], yg[:])      # ACT
            nc.vector.tensor_mul(p_g[:], XBg, XIBg)  # DVE 2x

            for j in range(G):
                b = g * G + j
                st = b == 0
                sp = b == NB - 1
                nc.tensor.matmul(S_r[:], identb[:], XB[:, b, :], start=st, stop=sp)
                nc.tensor.matmul(S_i[:], identb[:], XIB[:, b, :], start=st, stop=sp)
                nc.tensor.matmul(S_rr[:], identb[:], sq_r[:, j, :], start=st, stop=sp)
                nc.tensor.matmul(S_ii[:], identb[:], sq_i[:, j, :], start=st, stop=sp)
                nc.tensor.matmul(S_ri[:], identb[:], p_g[:, j, :], start=st, stop=sp)
        return XB, XIB

    def extracts(t, k, h):
        """ACT: PSUM stat sums -> fp16 mean/var tiles (pair-indexed h=t%2)."""
        if h == 0:
            ex_tiles[k] = {
                nm: epool.tile([P, 2, F], f16, tag=nm, name=f"{nm}_{k}")
                for nm in ("mu_r", "mu_i", "Vrr", "Vii", "Vri")
            }
        e = ex_tiles[k]
        nc.scalar.activation(e["mu_r"][:, h, :], S_r[:], ACT.Copy, scale=inv16)
        nc.scalar.activation(e["mu_i"][:, h, :], S_i[:], ACT.Copy, scale=inv16)
        nc.scalar.activation(e["Vrr"][:, h, :], S_rr[:], ACT.Copy, bias=EPS, scale=inv16)
        nc.scalar.activation(e["Vii"][:, h, :], S_ii[:], ACT.Copy, bias=EPS, scale=inv16)
        nc.scalar.activation(e["Vri"][:, h, :], S_ri[:], ACT.Copy, scale=inv16)

    par_tiles = {}

    def load_params(k):
        """Prefetch gamma/beta f32 for both tiles of pair k."""
        t0 = 2 * k
        gr = gpool.tile([P, 2, F], f32, tag="grr", name=f"grr{k}")
        gi = gpool.tile([P, 2, F], f32, tag="gri", name=f"gri{k}")
        gg = gpool.tile([P, 2, F], f32, tag="gii", name=f"gii{k}")
        bt = gpool.tile([P, 2, F], f32, tag="bet", name=f"bet{k}")
        for h in range(2):
            nc.sync.dma_start(gr[:, h, :], grr_v[t0 + h])
            nc.sync.dma_start(gi[:, h, :], gri_v[t0 + h])
            nc.sync.dma_start(gg[:, h, :], gii_v[t0 + h])
            nc.sync.dma_start(bt[:, h, :], bet_v[t0 + h])
        par_tiles[k] = (gr, gi, gg, bt)

    def coef(k):
        """Per tile-pair coefficient math, fp16, FD=1024 ops."""
        e = ex_tiles[k]
        mu_r, mu_i = e["mu_r"], e["mu_i"]
        Vrr, Vii, Vri = e["Vrr"], e["Vii"], e["Vri"]
        gr, gi, gg, bt = par_tiles[k]

        cp = lambda tag: cpool.tile([P, 2, F], f16, tag=tag, name=f"{tag}{k}", bufs=1)
        gr16 = cp("gr16"); nc.vector.tensor_copy(gr16[:], gr[:])
        gi16 = cp("gi16"); nc.vector.tensor_copy(gi16[:], gi[:])
        gg16 = cp("gg16"); nc.vector.tensor_copy(gg16[:], gg[:])
        bt16 = cp("bt16"); nc.vector.tensor_copy(bt16[:], bt[:])

        mr2 = cp("s0")
        nc.scalar.square(mr2[:], mu_r[:])
        nc.vector.tensor_sub(Vrr[:], Vrr[:], mr2[:])
        mi2 = cp("s1")
        nc.scalar.square(mi2[:], mu_i[:])
        nc.vector.tensor_sub(Vii[:], Vii[:], mi2[:])
        mri = cp("s2")
        nc.vector.tensor_mul(mri[:], mu_r[:], mu_i[:])
        nc.vector.tensor_sub(Vri[:], Vri[:], mri[:])

        tau = cp("s3")
        nc.vector.tensor_add(tau[:], Vrr[:], Vii[:])
        det = cp("s4")
        nc.vector.tensor_mul(det[:], Vrr[:], Vii[:])
        vri2 = cp("s0")
        nc.scalar.square(vri2[:], Vri[:])
        nc.vector.tensor_sub(det[:], det[:], vri2[:])

        s_s = cp("s5")
        nc.scalar.sqrt(s_s[:], det[:])
        nc.vector.scalar_tensor_tensor(
            tau[:], s_s[:], 2.0, tau[:], mybir.AluOpType.mult, mybir.AluOpType.add
        )
        t_t = cp("s6")
        nc.scalar.sqrt(t_t[:], tau[:])
        inv = cp("s8")
        for h in range(2):
            st32 = cpool.tile([P, F], f32, tag="st32", name=f"st32_{k}_{h}", bufs=1)
            nc.vector.tensor_mul(st32[:], s_s[:, h, :], t_t[:, h, :])
            inv32 = cpool.tile([P, F], f32, tag="inv32", name=f"inv32_{k}_{h}", bufs=1)
            nc.vector.reciprocal_approx_fast(inv32[:], st32[:])
            nc.vector.tensor_copy(inv[:, h, :], inv32[:])

        # W matrix (Wri holds +Vri*inv; signs folded below)
        nc.vector.tensor_add(Vii[:], Vii[:], s_s[:])
        Wrr = cp("s3")
        nc.vector.tensor_mul(Wrr[:], Vii[:], inv[:])
        nc.vector.tensor_add(Vrr[:], Vrr[:], s_s[:])
        Wii = cp("s4")
        nc.vector.tensor_mul(Wii[:], Vrr[:], inv[:])
        Wri = cp("s5")
        nc.vector.tensor_mul(Wri[:], Vri[:], inv[:])

        co = {nm: cpool.tile([P, 2, F], f16, tag=nm, name=f"{nm}_{k}")
              for nm in ("a1", "a2", "b1", "b2", "a0", "b0")}
        m2 = cp("s6")
        nc.vector.tensor_mul(co["a1"][:], gr16[:], Wrr[:])
        nc.vector.tensor_mul(m2[:], gi16[:], Wri[:])
        nc.vector.tensor_sub(co["a1"][:], co["a1"][:], m2[:])

        m4 = cp("s7")
        nc.vector.tensor_mul(co["a2"][:], gi16[:], Wii[:])
        nc.vector.tensor_mul(m4[:], gr16[:], Wri[:])
        nc.vector.tensor_sub(co["a2"][:], co["a2"][:], m4[:])

        m6 = cp("s8")
        nc.vector.tensor_mul(co["b1"][:], gi16[:], Wrr[:])
        nc.vector.tensor_mul(m6[:], gg16[:], Wri[:])
        nc.vector.tensor_sub(co["b1"][:], co["b1"][:], m6[:])

        nc.vector.tensor_mul(co["b2"][:], gg16[:], Wii[:])
        nc.vector.tensor_sub(co["b2"][:], co["b2"][:], m2[:])

        n1 = cp("s0")
        nc.vector.tensor_mul(n1[:], co["a1"][:], mu_r[:])
        nc.vector.tensor_sub(co["a0"][:], bt16[:], n1[:])
        n2 = cp("s1")
        nc.vector.tensor_mul(n2[:], co["a2"][:], mu_i[:])
        nc.vector.tensor_sub(co["a0"][:], co["a0"][:], n2[:])

        n3 = cp("s2")
        nc.vector.tensor_mul(n3[:], co["b1"][:], mu_r[:])
        nc.vector.tensor_sub(co["b0"][:], bt16[:], n3[:])
        n4 = cp("s6")
        nc.vector.tensor_mul(n4[:], co["b2"][:], mu_i[:])
        nc.vector.tensor_sub(co["b0"][:], co["b0"][:], n4[:])

        coef_tiles[k] = co

    nfin = [0]

    def pass2(t, XB, XIB):
        """DVE muls + TensorE sum streams + split copyouts + DMA out."""
        k, h = t // 2, t % 2
        co = coef_tiles[k]
        a1 = co["a1"][:, h, :]
        a2 = co["a2"][:, h, :]
        b1 = co["b1"][:, h, :]
        b2 = co["b2"][:, h, :]
        a0 = co["a0"][:, h, :]
        b0 = co["b0"][:, h, :]
        for q in range(NB // QB):
            b0_ = q * QB
            XBq = XB[:, b0_:b0_ + QB, :]
            XIBq = XIB[:, b0_:b0_ + QB, :]
            U = wpool.tile([P, QB, F], f16, tag="U", name=f"U{t}_{q}")
            nc.vector.tensor_mul(U[:], XBq, bcast_free(a1, QB))
            V = wpool.tile([P, QB, F], f16, tag="V", name=f"V{t}_{q}")
            nc.vector.tensor_mul(V[:], XIBq, bcast_free(a2, QB))
            U2 = wpool.tile([P, QB, F], f16, tag="U2", name=f"U2{t}_{q}")
            nc.vector.tensor_mul(U2[:], XBq, bcast_free(b1, QB))
            V2 = wpool.tile([P, QB, F], f16, tag="V2", name=f"V2{t}_{q}")
            nc.vector.tensor_mul(V2[:], XIBq, bcast_free(b2, QB))

            o_r = opool.tile([P, QB, F], f16, tag="or", name=f"or{t}_{q}")
            o_i = opool.tile([P, QB, F], f16, tag="oi", name=f"oi{t}_{q}")
            for j in range(QB):
                for comp, (Uc, Vc, cc, oc) in enumerate(
                    ((U, V, a0, o_r), (U2, V2, b0, o_i))
                ):
                    PS = psum.tile([P, F], f32, tag="PS",
                                   name=f"PS{t}_{q}_{j}_{comp}", bufs=3)
                    nc.tensor.matmul(PS[:], identb[:], Uc[:, j, :], start=True, stop=False)
                    nc.tensor.matmul(PS[:], identb[:], Vc[:, j, :], start=False, stop=False)
                    nc.tensor.matmul(PS[:], identb[:], cc, start=False, stop=True)
                    if nfin[0] % 16 < dve_copyout_share:
                        nc.vector.tensor_copy(oc[:, j, :], PS[:])
                    else:
                        nc.scalar.copy(oc[:, j, :], PS[:])
                    nfin[0] += 1
            # out-DMAs via ScalarE HWDGE: keeps the sync queue pure input
            # loads (no head-of-line blocking of the next pair's x loads)
            nc.scalar.dma_start(or_v[q, t], o_r[:])
            nc.scalar.dma_start(oi_v[q, t], o_i[:])

    # ---- main schedule: pairs of tiles, software-pipelined ----
    global S_r, S_i, S_rr, S_ii, S_ri
    S_r = psum.tile([P, F], f32, tag="S_r")
    S_i = psum.tile([P, F], f32, tag="S_i")
    S_rr = psum.tile([P, F], f32, tag="S_rr")
    S_ii = psum.tile([P, F], f32, tag="S_ii")
    S_ri = psum.tile([P, F], f32, tag="S_ri")

    xb_prev = {}
    for k in range(NPAIR):
        t0, t1 = 2 * k, 2 * k + 1
        xb0 = pass1(t0)
        extracts(t0, k, 0)
        load_params(k)
        xb1 = pass1(t1)
        extracts(t1, k, 1)
        coef(k)
        pass2(t0, *xb0)
        pass2(t1, *xb1)


def build_nc(npos: int = NPOS_FULL, dve_copyout_share: int = 3) -> bacc.Bacc:
    nc = bacc.Bacc("TRN2", target_bir_lowering=False, debug=False)
    with tile.TileContext(nc) as tc:
        with ExitStack() as ctx:
            _emit(nc, ctx, tc, npos, dve_copyout_share=dve_copyout_share)
    nc.compile()
    return nc


_cache: dict = {}


def _get_nc(npos: int = NPOS_FULL, dve_copyout_share: int = 3) -> bacc.Bacc:
    key = (npos, dve_copyout_share)
    if key not in _cache:
        _cache[key] = build_nc(npos, dve_copyout_share)
    return _cache[key]


def make_in_maps(x_real, x_imag, gamma_rr, gamma_ri, gamma_ii, beta):
    """Shard channels across cores; returns per-core input dicts."""
    in_maps = []
    for c in range(N_CORES):
        sl = slice(c * C_LOC, (c + 1) * C_LOC)
        in_maps.append(
            {
                "xr": np.ascontiguousarray(x_real[:, sl]).reshape(NB, -1),
                "xi": np.ascontiguousarray(x_imag[:, sl]).reshape(NB, -1),
                "grr": np.ascontiguousarray(gamma_rr[sl]).reshape(-1),
                "gri": np.ascontiguousarray(gamma_ri[sl]).reshape(-1),
                "gii": np.ascontiguousarray(gamma_ii[sl]).reshape(-1),
                "bet": np.ascontiguousarray(beta[sl]).reshape(-1),
            }
        )
    return in_maps


def assemble_output(results) -> np.ndarray:
    """Gather per-core fp16 (re, im) outputs into the full complex64 array."""
    out = np.empty((NB, C_FULL, HW), dtype=np.complex64)
    for c in range(N_CORES):
        o_r = np.asarray(results[c]["outr"])  # [NB, NPOS] fp16
        o_i = np.asarray(results[c]["outi"])
        sl = slice(c * C_LOC, (c + 1) * C_LOC)
        out.real[:, sl] = o_r.astype(np.float32).reshape(NB, C_LOC, HW)
        out.imag[:, sl] = o_i.astype(np.float32).reshape(NB, C_LOC, HW)
    return out.reshape(NB, C_FULL, 256, 256)


def kernel(x_real, x_imag, gamma_rr, gamma_ri, gamma_ii, beta) -> np.ndarray:
    x_real = np.asarray(x_real, dtype=np.float32)
    x_imag = np.asarray(x_imag, dtype=np.float32)
    gamma_rr = np.asarray(gamma_rr, dtype=np.float32)
    gamma_ri = np.asarray(gamma_ri, dtype=np.float32)
    gamma_ii = np.asarray(gamma_ii, dtype=np.float32)
    beta = np.asarray(beta, dtype=np.float32)

    nc = _get_nc(NPOS_FULL)
    in_maps = make_in_maps(x_real, x_imag, gamma_rr, gamma_ri, gamma_ii, beta)
    res = run_bass_kernel_spmd(nc, in_maps, core_ids=list(range(N_CORES)))
    return assemble_output(res.results)


# revision 10
# speedup vs baseline: 1.1991x; 1.1991x over previous
"""Complex batch-norm Trainium2 kernel (nn_ComplexBatchNormal).

Full inputs: x_real/x_imag [16, 32, 256, 256] f32, params [32, 256, 256] f32.
Output: complex64 [16, 32, 256, 256].

Sharding: channels C=32 split across 8 cores (4 channels each) -> fully local
batch statistics per core, no collectives.

Per-core algorithm (positions N = 4*256*256 = 262144, batch B = 16, 4 tiles
of [128, 512] positions):
  pass 1: load x f32, cast to fp16 (ACT for xr, DVE for xi), squares on ACT,
          product on DVE; 5 stats accumulated over B via TensorE identity
          matmuls into 5 PSUM banks.
  coef:   per tile-PAIR (FD=1024 ops), fp16 throughout: analytic inverse-sqrt
          of the 2x2 covariance folded with gamma/beta/mu into 6 fp16
          coefficients a1,a2,a0,b1,b2,b0.
  pass 2: DVE muls U=a1*xr, V=a2*xi (fp16 2x, quarter-batch granularity);
          U+V+bias summed on TensorE into PSUM (3 streams/comp/sample);
          per-sample PSUM->SBUF fp16 copyouts split DVE/ACT; outputs are two
          fp16 DRAM tensors (re, im) upcast + combined on host.
"""

import sys

if "/opt/trn_rl_repo" not in sys.path:
    sys.path.insert(0, "/opt/trn_rl_repo")

from contextlib import ExitStack

import numpy as np

import concourse.bacc as bacc
import concourse.bass as bass
import concourse.tile as tile
from concourse import masks, mybir
from concourse.bass_utils import run_bass_kernel_spmd

P = 128          # SBUF partitions
F = 512          # free-dim positions per tile (= one PSUM bank of f32)
NB = 16          # batch size
G = 4            # batch-samples per load group
QB = 2           # batch-samples per pass-2 mul/out group
EPS = 1e-5
N_CORES = 8
C_FULL = 32
C_LOC = C_FULL // N_CORES  # 4 channels per core
HW = 256 * 256
NPOS_FULL = C_LOC * HW     # 262144 positions per core

f32 = mybir.dt.float32
f16 = mybir.dt.float16

ACT = mybir.ActivationFunctionType


def bcast_free(ap: bass.AP, n: int) -> bass.AP:
    """View [P, F] as [P, n, F] with the middle dim broadcast (step 0)."""
    return bass.AP(tensor=ap.tensor, offset=ap.offset, ap=[ap.ap[0], [0, n], ap.ap[1]])


def _emit(nc: bacc.Bacc, ctx: ExitStack, tc: "tile.TileContext", npos: int,
          dve_copyout_share: int = 3):
    NT = npos // (P * F)
    assert NT * P * F == npos and NT % 2 == 0

    xr_d = nc.dram_tensor("xr", [NB, npos], f32, kind="ExternalInput")
    xi_d = nc.dram_tensor("xi", [NB, npos], f32, kind="ExternalInput")
    grr_d = nc.dram_tensor("grr", [npos], f32, kind="ExternalInput")
    gri_d = nc.dram_tensor("gri", [npos], f32, kind="ExternalInput")
    gii_d = nc.dram_tensor("gii", [npos], f32, kind="ExternalInput")
    bet_d = nc.dram_tensor("bet", [npos], f32, kind="ExternalInput")
    or_d = nc.dram_tensor("outr", [NB, npos], f16, kind="ExternalOutput")
    oi_d = nc.dram_tensor("outi", [NB, npos], f16, kind="ExternalOutput")

    xr_gv = xr_d.ap().rearrange("(g q) (t p f) -> g t p q f", q=G, p=P, f=F)
    xi_gv = xi_d.ap().rearrange("(g q) (t p f) -> g t p q f", q=G, p=P, f=F)
    grr_v = grr_d.ap().rearrange("(t p f) -> t p f", p=P, f=F)
    gri_v = gri_d.ap().rearrange("(t p f) -> t p f", p=P, f=F)
    gii_v = gii_d.ap().rearrange("(t p f) -> t p f", p=P, f=F)
    bet_v = bet_d.ap().rearrange("(t p f) -> t p f", p=P, f=F)
    or_v = or_d.ap().rearrange("(q b) (t p f) -> q t p b f", b=QB, p=P, f=F)
    oi_v = oi_d.ap().rearrange("(q b) (t p f) -> q t p b f", b=QB, p=P, f=F)

    singles = ctx.enter_context(tc.tile_pool(name="singles", bufs=1))
    xpool = ctx.enter_context(tc.tile_pool(name="x", bufs=2))      # f32 staging
    xbpool = ctx.enter_context(tc.tile_pool(name="xb", bufs=2))    # fp16 resident
    sqpool = ctx.enter_context(tc.tile_pool(name="sq", bufs=1))
    gpool = ctx.enter_context(tc.tile_pool(name="g", bufs=1))
    epool = ctx.enter_context(tc.tile_pool(name="ex", bufs=1))     # stat extracts (pairs)
    cpool = ctx.enter_context(tc.tile_pool(name="coef", bufs=2))   # coef scratch (pairs)
    wpool = ctx.enter_context(tc.tile_pool(name="w", bufs=1))      # U/V tiles
    opool = ctx.enter_context(tc.tile_pool(name="o", bufs=2))      # fp16 out tiles
    psum = ctx.enter_context(tc.tile_pool(name="ps", bufs=1, space="PSUM"))

    ident = singles.tile([P, P], f32)
    masks.make_identity(nc, ident[:])
    identb = singles.tile([P, P], f16)
    nc.scalar.copy(identb[:], ident[:])

    inv16 = 1.0 / NB

    NPAIR = NT // 2

    # per-pair persistent handles
    ex_tiles = {}
    coef_tiles = {}

    def pass1(t):
        """Load + cast + squares + product + stat matmuls for tile t."""
        XB = xbpool.tile([P, NB, F], f16, tag="XB", name=f"XB{t}")
        XIB = xbpool.tile([P, NB, F], f16, tag="XIB", name=f"XIB{t}")
        for g in range(NB // G):
            xg = xpool.tile([P, G, F], f32, tag="xr", name=f"xr{t}_{g}")
            nc.sync.dma_start(xg[:], xr_gv[g, t])
            yg = xpool.tile([P, G, F], f32, tag="xi", name=f"xi{t}_{g}")
            nc.sync.dma_start(yg[:], xi_gv[g, t])

            XBg = XB[:, g * G:(g + 1) * G, :]
            XIBg = XIB[:, g * G:(g + 1) * G, :]
            nc.scalar.copy(XBg, xg[:])            # ACT cast
            nc.vector.tensor_copy(XIBg, yg[:])    # DVE cast (2x-2p)

            sq_r = sqpool.tile([P, G, F], f16, tag="sqr", name=f"sqr{t}_{g}")
            sq_i = sqpool.tile([P, G, F], f16, tag="sqi", name=f"sqi{t}_{g}")
            p_g = sqpool.tile([P, G, F], f16, tag="pg", name=f"pg{t}_{g}")
            nc.scalar.square(sq_r[:], xg[:])      # ACT
            nc.scalar.square(sq_i[:], yg[:])      # ACT
            nc.vector.tensor_mul(p_g[:], XBg, XIBg)  # DVE 2x

            for j in range(G):
                b = g * G + j
                st = b == 0
                sp = b == NB - 1
                nc.tensor.matmul(S_r[:], identb[:], XB[:, b, :], start=st, stop=sp)
                nc.tensor.matmul(S_i[:], identb[:], XIB[:, b, :], start=st, stop=sp)
                nc.tensor.matmul(S_rr[:], identb[:], sq_r[:, j, :], start=st, stop=sp)
                nc.tensor.matmul(S_ii[:], identb[:], sq_i[:, j, :], start=st, stop=sp)
                nc.tensor.matmul(S_ri[:], identb[:], p_g[:, j, :], start=st, stop=sp)
        return XB, XIB

    def extracts(t, k, h):
        """ACT: PSUM stat sums -> fp16 mean/var tiles (pair-indexed h=t%2)."""
        if h == 0:
            ex_tiles[k] = {
                nm: epool.tile([P, 2, F], f16, tag=nm, name=f"{nm}_{k}")
                for nm in ("mu_r", "mu_i", "Vrr", "Vii", "Vri")
            }
        e = ex_tiles[k]
        nc.scalar.activation(e["mu_r"][:, h, :], S_r[:], ACT.Copy, scale=inv16)
        nc.scalar.activation(e["mu_i"][:, h, :], S_i[:], ACT.Copy, scale=inv16)
        nc.scalar.activation(e["Vrr"][:, h, :], S_rr[:], ACT.Copy, bias=EPS, scale=inv16)
        nc.scalar.activation(e["Vii"][:, h, :], S_ii[:], ACT.Copy, bias=EPS, scale=inv16)
        nc.scalar.activation(e["Vri"][:, h, :], S_ri[:], ACT.Copy, scale=inv16)

    par_tiles = {}

    def load_params(k):
        """Prefetch gamma/beta f32 for both tiles of pair k."""
        t0 = 2 * k
        gr = gpool.tile([P, 2, F], f32, tag="grr", name=f"grr{k}")
        gi = gpool.tile([P, 2, F], f32, tag="gri", name=f"gri{k}")
        gg = gpool.tile([P, 2, F], f32, tag="gii", name=f"gii{k}")
        bt = gpool.tile([P, 2, F], f32, tag="bet", name=f"bet{k}")
        for h in range(2):
            nc.sync.dma_start(gr[:, h, :], grr_v[t0 + h])
            nc.sync.dma_start(gi[:, h, :], gri_v[t0 + h])
            nc.sync.dma_start(gg[:, h, :], gii_v[t0 + h])
            nc.sync.dma_start(bt[:, h, :], bet_v[t0 + h])
        par_tiles[k] = (gr, gi, gg, bt)

    def coef(k):
        """Per tile-pair coefficient math, fp16, FD=1024 ops."""
        e = ex_tiles[k]
        mu_r, mu_i = e["mu_r"], e["mu_i"]
        Vrr, Vii, Vri = e["Vrr"], e["Vii"], e["Vri"]
        gr, gi, gg, bt = par_tiles[k]

        cp = lambda tag: cpool.tile([P, 2, F], f16, tag=tag, name=f"{tag}{k}", bufs=1)
        gr16 = cp("gr16"); nc.vector.tensor_copy(gr16[:], gr[:])
        gi16 = cp("gi16"); nc.vector.tensor_copy(gi16[:], gi[:])
        gg16 = cp("gg16"); nc.vector.tensor_copy(gg16[:], gg[:])
        bt16 = cp("bt16"); nc.vector.tensor_copy(bt16[:], bt[:])

        mr2 = cp("s0")
        nc.scalar.square(mr2[:], mu_r[:])
        nc.vector.tensor_sub(Vrr[:], Vrr[:], mr2[:])
        mi2 = cp("s1")
        nc.scalar.square(mi2[:], mu_i[:])
        nc.vector.tensor_sub(Vii[:], Vii[:], mi2[:])
        mri = cp("s2")
        nc.vector.tensor_mul(mri[:], mu_r[:], mu_i[:])
        nc.vector.tensor_sub(Vri[:], Vri[:], mri[:])

        tau = cp("s3")
        nc.vector.tensor_add(tau[:], Vrr[:], Vii[:])
        det = cp("s4")
        nc.vector.tensor_mul(det[:], Vrr[:], Vii[:])
        vri2 = cp("s0")
        nc.scalar.square(vri2[:], Vri[:])
        nc.vector.tensor_sub(det[:], det[:], vri2[:])

        s_s = cp("s5")
        nc.scalar.sqrt(s_s[:], det[:])
        nc.vector.scalar_tensor_tensor(
            tau[:], s_s[:], 2.0, tau[:], mybir.AluOpType.mult, mybir.AluOpType.add
        )
        t_t = cp("s6")
        nc.scalar.sqrt(t_t[:], tau[:])
        inv = cp("s8")
        for h in range(2):
            st32 = cpool.tile([P, F], f32, tag="st32", name=f"st32_{k}_{h}", bufs=1)
            nc.vector.tensor_mul(st32[:], s_s[:, h, :], t_t[:, h, :])
            inv32 = cpool.tile([P, F], f32, tag="inv32", name=f"inv32_{k}_{h}", bufs=1)
            nc.vector.reciprocal_approx_fast(inv32[:], st32[:])
            nc.vector.tensor_copy(inv[:, h, :], inv32[:])

        # W matrix (Wri holds +Vri*inv; signs folded below)
        nc.vector.tensor_add(Vii[:], Vii[:], s_s[:])
        Wrr = cp("s3")
        nc.vector.tensor_mul(Wrr[:], Vii[:], inv[:])
        nc.vector.tensor_add(Vrr[:], Vrr[:], s_s[:])
        Wii = cp("s4")
        nc.vector.tensor_mul(Wii[:], Vrr[:], inv[:])
        Wri = cp("s5")
        nc.vector.tensor_mul(Wri[:], Vri[:], inv[:])

        co = {nm: cpool.tile([P, 2, F], f16, tag=nm, name=f"{nm}_{k}")
              for nm in ("a1", "a2", "b1", "b2", "a0", "b0")}
        m2 = cp("s6")
        nc.vector.tensor_mul(co["a1"][:], gr16[:], Wrr[:])
        nc.vector.tensor_mul(m2[:], gi16[:], Wri[:])
        nc.vector.tensor_sub(co["a1"][:], co["a1"][:], m2[:])

        m4 = cp("s7")
        nc.vector.tensor_mul(co["a2"][:], gi16[:], Wii[:])
        nc.vector.tensor_mul(m4[:], gr16[:], Wri[:])
        nc.vector.tensor_sub(co["a2"][:], co["a2"][:], m4[:])

        m6 = cp("s8")
        nc.vector.tensor_mul(co["b1"][:], gi16[:], Wrr[:])
        nc.vector.tensor_mul(m6[:], gg16[:], Wri[:])
        nc.vector.tensor_sub(co["b1"][:], co["b1"][:], m6[:])

        nc.vector.tensor_mul(co["b2"][:], gg16[:], Wii[:])
        nc.vector.tensor_sub(co["b2"][:], co["b2"][:], m2[:])

        n1 = cp("s0")
        nc.vector.tensor_mul(n1[:], co["a1"][:], mu_r[:])
        nc.vector.tensor_sub(co["a0"][:], bt16[:], n1[:])
        n2 = cp("s1")
        nc.vector.tensor_mul(n2[:], co["a2"][:], mu_i[:])
        nc.vector.tensor_sub(co["a0"][:], co["a0"][:], n2[:])

        n3 = cp("s2")
        nc.vector.tensor_mul(n3[:], co["b1"][:], mu_r[:])
        nc.vector.tensor_sub(co["b0"][:], bt16[:], n3[:])
        n4 = cp("s6")
        nc.vector.tensor_mul(n4[:], co["b2"][:], mu_i[:])
        nc.vector.tensor_sub(co["b0"][:], co["b0"][:], n4[:])

        coef_tiles[k] = co

    nfin = [0]

    def pass2(t, XB, XIB):
        """DVE muls + TensorE sum streams + split copyouts + DMA out."""
        k, h = t // 2, t % 2
        co = coef_tiles[k]
        a1 = co["a1"][:, h, :]
        a2 = co["a2"][:, h, :]
        b1 = co["b1"][:, h, :]
        b2 = co["b2"][:, h, :]
        a0 = co["a0"][:, h, :]
        b0 = co["b0"][:, h, :]
        for q in range(NB // QB):
            b0_ = q * QB
            XBq = XB[:, b0_:b0_ + QB, :]
            XIBq = XIB[:, b0_:b0_ + QB, :]
            U = wpool.tile([P, QB, F], f16, tag="U", name=f"U{t}_{q}")
            nc.vector.tensor_mul(U[:], XBq, bcast_free(a1, QB))
            V = wpool.tile([P, QB, F], f16, tag="V", name=f"V{t}_{q}")
            nc.vector.tensor_mul(V[:], XIBq, bcast_free(a2, QB))
            U2 = wpool.tile([P, QB, F], f16, tag="U2", name=f"U2{t}_{q}")
            nc.vector.tensor_mul(U2[:], XBq, bcast_free(b1, QB))
            V2 = wpool.tile([P, QB, F], f16, tag="V2", name=f"V2{t}_{q}")
            nc.vector.tensor_mul(V2[:], XIBq, bcast_free(b2, QB))

            o_r = opool.tile([P, QB, F], f16, tag="or", name=f"or{t}_{q}")
            o_i = opool.tile([P, QB, F], f16, tag="oi", name=f"oi{t}_{q}")
            for j in range(QB):
                for comp, (Uc, Vc, cc, oc) in enumerate(
                    ((U, V, a0, o_r), (U2, V2, b0, o_i))
                ):
                    PS = psum.tile([P, F], f32, tag="PS",
                                   name=f"PS{t}_{q}_{j}_{comp}", bufs=3)
                    nc.tensor.matmul(PS[:], identb[:], Uc[:, j, :], start=True, stop=False)
                    nc.tensor.matmul(PS[:], identb[:], Vc[:, j, :], start=False, stop=False)
                    nc.tensor.matmul(PS[:], identb[:], cc, start=False, stop=True)
                    if nfin[0] % 16 < dve_copyout_share:
                        nc.vector.tensor_copy(oc[:, j, :], PS[:])
                    else:
                        nc.scalar.copy(oc[:, j, :], PS[:])
                    nfin[0] += 1
            nc.sync.dma_start(or_v[q, t], o_r[:])
            nc.sync.dma_start(oi_v[q, t], o_i[:])

    # ---- main schedule: pairs of tiles, software-pipelined ----
    global S_r, S_i, S_rr, S_ii, S_ri
    S_r = psum.tile([P, F], f32, tag="S_r")
    S_i = psum.tile([P, F], f32, tag="S_i")
    S_rr = psum.tile([P, F], f32, tag="S_rr")
    S_ii = psum.tile([P, F], f32, tag="S_ii")
    S_ri = psum.tile([P, F], f32, tag="S_ri")

    xb_prev = {}
    for k in range(NPAIR):
        t0, t1 = 2 * k, 2 * k + 1
        xb0 = pass1(t0)
        extracts(t0, k, 0)
        load_params(k)
        xb1 = pass1(t1)
        extracts(t1, k, 1)
        coef(k)
        pass2(t0, *xb0)
        pass2(t1, *xb1)


def build_nc(npos: int = NPOS_FULL, dve_copyout_share: int = 3) -> bacc.Bacc:
    nc = bacc.Bacc("TRN2", target_bir_lowering=False, debug=False)
    with tile.TileContext(nc) as tc:
        with ExitStack() as ctx:
            _emit(nc, ctx, tc, npos, dve_copyout_share=dve_copyout_share)
    nc.compile()
    return nc


_cache: dict = {}


def _get_nc(npos: int = NPOS_FULL, dve_copyout_share: int = 3) -> bacc.Bacc:
    key = (npos, dve_copyout_share)
    if key not in _cache:
        _cache[key] = build_nc(npos, dve_copyout_share)
    return _cache[key]


def make_in_maps(x_real, x_imag, gamma_rr, gamma_ri, gamma_ii, beta):
    """Shard channels across cores; returns per-core input dicts."""
    in_maps = []
    for c in range(N_CORES):
        sl = slice(c * C_LOC, (c + 1) * C_LOC)
        in_maps.append(
            {
                "xr": np.ascontiguousarray(x_real[:, sl]).reshape(NB, -1),
                "xi": np.ascontiguousarray(x_imag[:, sl]).reshape(NB, -1),
                "grr": np.ascontiguousarray(gamma_rr[sl]).reshape(-1),
                "gri": np.ascontiguousarray(gamma_ri[sl]).reshape(-1),
                "gii": np.ascontiguousarray(gamma_ii[sl]).reshape(-1),
                "bet": np.ascontiguousarray(beta[sl]).reshape(-1),
            }
        )
    return in_maps


def assemble_output(results) -> np.ndarray:
    """Gather per-core fp16 (re, im) outputs into the full complex64 array."""
    out = np.empty((NB, C_FULL, HW), dtype=np.complex64)
    for c in range(N_CORES):
        o_r = np.asarray(results[c]["outr"])  # [NB, NPOS] fp16
        o_i = np.asarray(results[c]["outi"])
        sl = slice(c * C_LOC, (c + 1) * C_LOC)
        out.real[:, sl] = o_r.astype(np.float32).reshape(NB, C_LOC, HW)
        out.imag[:, sl] = o_i.astype(np.float32).reshape(NB, C_LOC, HW)
    return out.reshape(NB, C_FULL, 256, 256)


def kernel(x_real, x_imag, gamma_rr, gamma_ri, gamma_ii, beta) -> np.ndarray:
    x_real = np.asarray(x_real, dtype=np.float32)
    x_imag = np.asarray(x_imag, dtype=np.float32)
    gamma_rr = np.asarray(gamma_rr, dtype=np.float32)
    gamma_ri = np.asarray(gamma_ri, dtype=np.float32)
    gamma_ii = np.asarray(gamma_ii, dtype=np.float32)
    beta = np.asarray(beta, dtype=np.float32)

    nc = _get_nc(NPOS_FULL)
    in_maps = make_in_maps(x_real, x_imag, gamma_rr, gamma_ri, gamma_ii, beta)
    res = run_bass_kernel_spmd(nc, in_maps, core_ids=list(range(N_CORES)))
    return assemble_output(res.results)


# revision 12
# speedup vs baseline: 1.2111x; 1.0100x over previous
"""Complex batch-norm Trainium2 kernel (nn_ComplexBatchNormal).

Full inputs: x_real/x_imag [16, 32, 256, 256] f32, params [32, 256, 256] f32.
Output: complex64 [16, 32, 256, 256].

Sharding: channels C=32 split across 8 cores (4 channels each) -> fully local
batch statistics per core, no collectives.

Per-core algorithm (positions N = 4*256*256 = 262144, batch B = 16, 4 tiles
of [128, 512] positions):
  pass 1: load x f32, cast to fp16 (ACT for xr, DVE for xi), squares on ACT,
          product on DVE; 5 stats accumulated over B via TensorE identity
          matmuls into 5 PSUM banks.
  coef:   per tile-PAIR (FD=1024 ops), fp16 throughout: analytic inverse-sqrt
          of the 2x2 covariance folded with gamma/beta/mu into 6 fp16
          coefficients a1,a2,a0,b1,b2,b0.
  pass 2: DVE muls U=a1*xr, V=a2*xi (fp16 2x, quarter-batch granularity);
          U+V+bias summed on TensorE into PSUM (3 streams/comp/sample);
          per-sample PSUM->SBUF fp16 copyouts split DVE/ACT; outputs are two
          fp16 DRAM tensors (re, im) upcast + combined on host.
"""

import sys

if "/opt/trn_rl_repo" not in sys.path:
    sys.path.insert(0, "/opt/trn_rl_repo")

from contextlib import ExitStack

import numpy as np

import concourse.bacc as bacc
import concourse.bass as bass
import concourse.tile as tile
from concourse import masks, mybir
from concourse.bass_utils import run_bass_kernel_spmd

P = 128          # SBUF partitions
F = 512          # free-dim positions per tile (= one PSUM bank of f32)
NB = 16          # batch size
G = 4            # batch-samples per load group
QB = 2           # batch-samples per pass-2 mul/out group
EPS = 1e-5
N_CORES = 8
C_FULL = 32
C_LOC = C_FULL // N_CORES  # 4 channels per core
HW = 256 * 256
NPOS_FULL = C_LOC * HW     # 262144 positions per core

f32 = mybir.dt.float32
f16 = mybir.dt.float16

ACT = mybir.ActivationFunctionType


def bcast_free(ap: bass.AP, n: int) -> bass.AP:
    """View [P, F] as [P, n, F] with the middle dim broadcast (step 0)."""
    return bass.AP(tensor=ap.tensor, offset=ap.offset, ap=[ap.ap[0], [0, n], ap.ap[1]])


def _emit(nc: bacc.Bacc, ctx: ExitStack, tc: "tile.TileContext", npos: int,
          dve_copyout_share: int = 3):
    NT = npos // (P * F)
    assert NT * P * F == npos and NT % 2 == 0

    xr_d = nc.dram_tensor("xr", [NB, npos], f32, kind="ExternalInput")
    xi_d = nc.dram_tensor("xi", [NB, npos], f32, kind="ExternalInput")
    grr_d = nc.dram_tensor("grr", [npos], f32, kind="ExternalInput")
    gri_d = nc.dram_tensor("gri", [npos], f32, kind="ExternalInput")
    gii_d = nc.dram_tensor("gii", [npos], f32, kind="ExternalInput")
    bet_d = nc.dram_tensor("bet", [npos], f32, kind="ExternalInput")
    or_d = nc.dram_tensor("outr", [NB, npos], f16, kind="ExternalOutput")
    oi_d = nc.dram_tensor("outi", [NB, npos], f16, kind="ExternalOutput")

    xr_gv = xr_d.ap().rearrange("(g q) (t p f) -> g t p q f", q=G, p=P, f=F)
    xi_gv = xi_d.ap().rearrange("(g q) (t p f) -> g t p q f", q=G, p=P, f=F)
    grr_v = grr_d.ap().rearrange("(t p f) -> t p f", p=P, f=F)
    gri_v = gri_d.ap().rearrange("(t p f) -> t p f", p=P, f=F)
    gii_v = gii_d.ap().rearrange("(t p f) -> t p f", p=P, f=F)
    bet_v = bet_d.ap().rearrange("(t p f) -> t p f", p=P, f=F)
    or_v = or_d.ap().rearrange("(q b) (t p f) -> q t p b f", b=QB, p=P, f=F)
    oi_v = oi_d.ap().rearrange("(q b) (t p f) -> q t p b f", b=QB, p=P, f=F)

    singles = ctx.enter_context(tc.tile_pool(name="singles", bufs=1))
    xpool = ctx.enter_context(tc.tile_pool(name="x", bufs=2))      # f32 staging
    xbpool = ctx.enter_context(tc.tile_pool(name="xb", bufs=2))    # fp16 resident
    sqpool = ctx.enter_context(tc.tile_pool(name="sq", bufs=1))
    gpool = ctx.enter_context(tc.tile_pool(name="g", bufs=1))
    epool = ctx.enter_context(tc.tile_pool(name="ex", bufs=1))     # stat extracts (pairs)
    cpool = ctx.enter_context(tc.tile_pool(name="coef", bufs=2))   # coef scratch (pairs)
    wpool = ctx.enter_context(tc.tile_pool(name="w", bufs=1))      # U/V tiles
    opool = ctx.enter_context(tc.tile_pool(name="o", bufs=2))      # fp16 out tiles
    psum = ctx.enter_context(tc.tile_pool(name="ps", bufs=1, space="PSUM"))

    ident = singles.tile([P, P], f32)
    masks.make_identity(nc, ident[:])
    identb = singles.tile([P, P], f16)
    nc.scalar.copy(identb[:], ident[:])

    inv16 = 1.0 / NB

    NPAIR = NT // 2

    # per-pair persistent handles
    ex_tiles = {}
    coef_tiles = {}

    prefetched = {}

    def prefetch_loads(t, ngroups=2):
        """Issue the first x-load DMAs of tile t early (fills DMA idle during
        the coefficient phase; limited by xg buffer rotation depth)."""
        for g in range(ngroups):
            xg = xpool.tile([P, G, F], f32, tag="xr", name=f"xr{t}_{g}")
            nc.sync.dma_start(xg[:], xr_gv[g, t])
            yg = xpool.tile([P, G, F], f32, tag="xi", name=f"xi{t}_{g}")
            nc.sync.dma_start(yg[:], xi_gv[g, t])
            prefetched[(t, g)] = (xg, yg)

    def pass1(t):
        """Load + cast + squares + product + stat matmuls for tile t."""
        XB = xbpool.tile([P, NB, F], f16, tag="XB", name=f"XB{t}")
        XIB = xbpool.tile([P, NB, F], f16, tag="XIB", name=f"XIB{t}")
        for g in range(NB // G):
            if (t, g) in prefetched:
                xg, yg = prefetched.pop((t, g))
            else:
                xg = xpool.tile([P, G, F], f32, tag="xr", name=f"xr{t}_{g}")
                nc.sync.dma_start(xg[:], xr_gv[g, t])
                yg = xpool.tile([P, G, F], f32, tag="xi", name=f"xi{t}_{g}")
                nc.sync.dma_start(yg[:], xi_gv[g, t])

            XBg = XB[:, g * G:(g + 1) * G, :]
            XIBg = XIB[:, g * G:(g + 1) * G, :]
            nc.scalar.copy(XBg, xg[:])            # ACT cast
            nc.vector.tensor_copy(XIBg, yg[:])    # DVE cast (2x-2p)

            sq_r = sqpool.tile([P, G, F], f16, tag="sqr", name=f"sqr{t}_{g}")
            sq_i = sqpool.tile([P, G, F], f16, tag="sqi", name=f"sqi{t}_{g}")
            p_g = sqpool.tile([P, G, F], f16, tag="pg", name=f"pg{t}_{g}")
            if t < 2:
                # ramp phase: DVE is idle while DMA/ACT pace pass 1 -> square
                # on DVE from the fp16 tiles (2x mode)
                nc.vector.tensor_mul(sq_r[:], XBg, XBg)
                nc.vector.tensor_mul(sq_i[:], XIBg, XIBg)
            else:
                nc.scalar.square(sq_r[:], xg[:])      # ACT
                nc.scalar.square(sq_i[:], yg[:])      # ACT
            nc.vector.tensor_mul(p_g[:], XBg, XIBg)  # DVE 2x

            for j in range(G):
                b = g * G + j
                st = b == 0
                sp = b == NB - 1
                nc.tensor.matmul(S_r[:], identb[:], XB[:, b, :], start=st, stop=sp)
                nc.tensor.matmul(S_i[:], identb[:], XIB[:, b, :], start=st, stop=sp)
                nc.tensor.matmul(S_rr[:], identb[:], sq_r[:, j, :], start=st, stop=sp)
                nc.tensor.matmul(S_ii[:], identb[:], sq_i[:, j, :], start=st, stop=sp)
                nc.tensor.matmul(S_ri[:], identb[:], p_g[:, j, :], start=st, stop=sp)
        return XB, XIB

    def extracts(t, k, h):
        """ACT: PSUM stat sums -> fp16 mean/var tiles (pair-indexed h=t%2)."""
        if h == 0:
            ex_tiles[k] = {
                nm: epool.tile([P, 2, F], f16, tag=nm, name=f"{nm}_{k}")
                for nm in ("mu_r", "mu_i", "Vrr", "Vii", "Vri")
            }
        e = ex_tiles[k]
        nc.scalar.activation(e["mu_r"][:, h, :], S_r[:], ACT.Copy, scale=inv16)
        nc.scalar.activation(e["mu_i"][:, h, :], S_i[:], ACT.Copy, scale=inv16)
        nc.scalar.activation(e["Vrr"][:, h, :], S_rr[:], ACT.Copy, bias=EPS, scale=inv16)
        nc.scalar.activation(e["Vii"][:, h, :], S_ii[:], ACT.Copy, bias=EPS, scale=inv16)
        nc.scalar.activation(e["Vri"][:, h, :], S_ri[:], ACT.Copy, scale=inv16)

    par_tiles = {}

    def load_params(k):
        """Prefetch gamma/beta f32 for both tiles of pair k."""
        t0 = 2 * k
        gr = gpool.tile([P, 2, F], f32, tag="grr", name=f"grr{k}")
        gi = gpool.tile([P, 2, F], f32, tag="gri", name=f"gri{k}")
        gg = gpool.tile([P, 2, F], f32, tag="gii", name=f"gii{k}")
        bt = gpool.tile([P, 2, F], f32, tag="bet", name=f"bet{k}")
        for h in range(2):
            nc.sync.dma_start(gr[:, h, :], grr_v[t0 + h])
            nc.sync.dma_start(gi[:, h, :], gri_v[t0 + h])
            nc.sync.dma_start(gg[:, h, :], gii_v[t0 + h])
            nc.sync.dma_start(bt[:, h, :], bet_v[t0 + h])
        par_tiles[k] = (gr, gi, gg, bt)

    def coef(k):
        """Per tile-pair coefficient math, fp16, FD=1024 ops."""
        e = ex_tiles[k]
        mu_r, mu_i = e["mu_r"], e["mu_i"]
        Vrr, Vii, Vri = e["Vrr"], e["Vii"], e["Vri"]
        gr, gi, gg, bt = par_tiles[k]

        cp = lambda tag: cpool.tile([P, 2, F], f16, tag=tag, name=f"{tag}{k}", bufs=1)
        gr16 = cp("gr16"); nc.vector.tensor_copy(gr16[:], gr[:])
        gi16 = cp("gi16"); nc.vector.tensor_copy(gi16[:], gi[:])
        gg16 = cp("gg16"); nc.vector.tensor_copy(gg16[:], gg[:])
        bt16 = cp("bt16"); nc.vector.tensor_copy(bt16[:], bt[:])

        mr2 = cp("s0")
        nc.scalar.square(mr2[:], mu_r[:])
        nc.vector.tensor_sub(Vrr[:], Vrr[:], mr2[:])
        mi2 = cp("s1")
        nc.scalar.square(mi2[:], mu_i[:])
        nc.vector.tensor_sub(Vii[:], Vii[:], mi2[:])
        mri = cp("s2")
        nc.vector.tensor_mul(mri[:], mu_r[:], mu_i[:])
        nc.vector.tensor_sub(Vri[:], Vri[:], mri[:])

        tau = cp("s3")
        nc.vector.tensor_add(tau[:], Vrr[:], Vii[:])
        det = cp("s4")
        nc.vector.tensor_mul(det[:], Vrr[:], Vii[:])
        vri2 = cp("s0")
        nc.scalar.square(vri2[:], Vri[:])
        nc.vector.tensor_sub(det[:], det[:], vri2[:])

        s_s = cp("s5")
        nc.scalar.sqrt(s_s[:], det[:])
        nc.vector.scalar_tensor_tensor(
            tau[:], s_s[:], 2.0, tau[:], mybir.AluOpType.mult, mybir.AluOpType.add
        )
        t_t = cp("s6")
        nc.scalar.sqrt(t_t[:], tau[:])
        inv = cp("s8")
        for h in range(2):
            st32 = cpool.tile([P, F], f32, tag="st32", name=f"st32_{k}_{h}", bufs=1)
            nc.vector.tensor_mul(st32[:], s_s[:, h, :], t_t[:, h, :])
            inv32 = cpool.tile([P, F], f32, tag="inv32", name=f"inv32_{k}_{h}", bufs=1)
            nc.vector.reciprocal_approx_fast(inv32[:], st32[:])
            nc.vector.tensor_copy(inv[:, h, :], inv32[:])

        # W matrix (Wri holds +Vri*inv; signs folded below)
        nc.vector.tensor_add(Vii[:], Vii[:], s_s[:])
        Wrr = cp("s3")
        nc.vector.tensor_mul(Wrr[:], Vii[:], inv[:])
        nc.vector.tensor_add(Vrr[:], Vrr[:], s_s[:])
        Wii = cp("s4")
        nc.vector.tensor_mul(Wii[:], Vrr[:], inv[:])
        Wri = cp("s5")
        nc.vector.tensor_mul(Wri[:], Vri[:], inv[:])

        co = {nm: cpool.tile([P, 2, F], f16, tag=nm, name=f"{nm}_{k}")
              for nm in ("a1", "a2", "b1", "b2", "a0", "b0")}
        m2 = cp("s6")
        nc.vector.tensor_mul(co["a1"][:], gr16[:], Wrr[:])
        nc.vector.tensor_mul(m2[:], gi16[:], Wri[:])
        nc.vector.tensor_sub(co["a1"][:], co["a1"][:], m2[:])

        m4 = cp("s7")
        nc.vector.tensor_mul(co["a2"][:], gi16[:], Wii[:])
        nc.vector.tensor_mul(m4[:], gr16[:], Wri[:])
        nc.vector.tensor_sub(co["a2"][:], co["a2"][:], m4[:])

        m6 = cp("s8")
        nc.vector.tensor_mul(co["b1"][:], gi16[:], Wrr[:])
        nc.vector.tensor_mul(m6[:], gg16[:], Wri[:])
        nc.vector.tensor_sub(co["b1"][:], co["b1"][:], m6[:])

        nc.vector.tensor_mul(co["b2"][:], gg16[:], Wii[:])
        nc.vector.tensor_sub(co["b2"][:], co["b2"][:], m2[:])

        n1 = cp("s0")
        nc.vector.tensor_mul(n1[:], co["a1"][:], mu_r[:])
        nc.vector.tensor_sub(co["a0"][:], bt16[:], n1[:])
        n2 = cp("s1")
        nc.vector.tensor_mul(n2[:], co["a2"][:], mu_i[:])
        nc.vector.tensor_sub(co["a0"][:], co["a0"][:], n2[:])

        n3 = cp("s2")
        nc.vector.tensor_mul(n3[:], co["b1"][:], mu_r[:])
        nc.vector.tensor_sub(co["b0"][:], bt16[:], n3[:])
        n4 = cp("s6")
        nc.vector.tensor_mul(n4[:], co["b2"][:], mu_i[:])
        nc.vector.tensor_sub(co["b0"][:], co["b0"][:], n4[:])

        coef_tiles[k] = co

    nfin = [0]

    def pass2(t, XB, XIB):
        """DVE muls + TensorE sum streams + split copyouts + DMA out."""
        k, h = t // 2, t % 2
        co = coef_tiles[k]
        a1 = co["a1"][:, h, :]
        a2 = co["a2"][:, h, :]
        b1 = co["b1"][:, h, :]
        b2 = co["b2"][:, h, :]
        a0 = co["a0"][:, h, :]
        b0 = co["b0"][:, h, :]
        for q in range(NB // QB):
            b0_ = q * QB
            XBq = XB[:, b0_:b0_ + QB, :]
            XIBq = XIB[:, b0_:b0_ + QB, :]
            U = wpool.tile([P, QB, F], f16, tag="U", name=f"U{t}_{q}")
            nc.vector.tensor_mul(U[:], XBq, bcast_free(a1, QB))
            V = wpool.tile([P, QB, F], f16, tag="V", name=f"V{t}_{q}")
            nc.vector.tensor_mul(V[:], XIBq, bcast_free(a2, QB))
            U2 = wpool.tile([P, QB, F], f16, tag="U2", name=f"U2{t}_{q}")
            nc.vector.tensor_mul(U2[:], XBq, bcast_free(b1, QB))
            V2 = wpool.tile([P, QB, F], f16, tag="V2", name=f"V2{t}_{q}")
            nc.vector.tensor_mul(V2[:], XIBq, bcast_free(b2, QB))

            o_r = opool.tile([P, QB, F], f16, tag="or", name=f"or{t}_{q}")
            o_i = opool.tile([P, QB, F], f16, tag="oi", name=f"oi{t}_{q}")
            for j in range(QB):
                for comp, (Uc, Vc, cc, oc) in enumerate(
                    ((U, V, a0, o_r), (U2, V2, b0, o_i))
                ):
                    PS = psum.tile([P, F], f32, tag="PS",
                                   name=f"PS{t}_{q}_{j}_{comp}", bufs=3)
                    nc.tensor.matmul(PS[:], identb[:], Uc[:, j, :], start=True, stop=False)
                    nc.tensor.matmul(PS[:], identb[:], Vc[:, j, :], start=False, stop=False)
                    nc.tensor.matmul(PS[:], identb[:], cc, start=False, stop=True)
                    share = 0 if t == NT - 1 else dve_copyout_share
                    if nfin[0] % 16 < share:
                        nc.vector.tensor_copy(oc[:, j, :], PS[:])
                    else:
                        nc.scalar.copy(oc[:, j, :], PS[:])
                    nfin[0] += 1
            nc.sync.dma_start(or_v[q, t], o_r[:])
            nc.sync.dma_start(oi_v[q, t], o_i[:])

    # ---- main schedule: pairs of tiles, software-pipelined ----
    global S_r, S_i, S_rr, S_ii, S_ri
    S_r = psum.tile([P, F], f32, tag="S_r")
    S_i = psum.tile([P, F], f32, tag="S_i")
    S_rr = psum.tile([P, F], f32, tag="S_rr")
    S_ii = psum.tile([P, F], f32, tag="S_ii")
    S_ri = psum.tile([P, F], f32, tag="S_ri")

    xb_prev = {}
    for k in range(NPAIR):
        t0, t1 = 2 * k, 2 * k + 1
        xb0 = pass1(t0)
        extracts(t0, k, 0)
        load_params(k)
        xb1 = pass1(t1)
        extracts(t1, k, 1)
        if k + 1 < NPAIR:
            prefetch_loads(2 * (k + 1), ngroups=2)
        coef(k)
        pass2(t0, *xb0)
        pass2(t1, *xb1)


def build_nc(npos: int = NPOS_FULL, dve_copyout_share: int = 3) -> bacc.Bacc:
    nc = bacc.Bacc("TRN2", target_bir_lowering=False, debug=False)
    with tile.TileContext(nc) as tc:
        with ExitStack() as ctx:
            _emit(nc, ctx, tc, npos, dve_copyout_share=dve_copyout_share)
    nc.compile()
    return nc


_cache: dict = {}


def _get_nc(npos: int = NPOS_FULL, dve_copyout_share: int = 3) -> bacc.Bacc:
    key = (npos, dve_copyout_share)
    if key not in _cache:
        _cache[key] = build_nc(npos, dve_copyout_share)
    return _cache[key]


def make_in_maps(x_real, x_imag, gamma_rr, gamma_ri, gamma_ii, beta):
    """Shard channels across cores; returns per-core input dicts."""
    in_maps = []
    for c in range(N_CORES):
        sl = slice(c * C_LOC, (c + 1) * C_LOC)
        in_maps.append(
            {
                "xr": np.ascontiguousarray(x_real[:, sl]).reshape(NB, -1),
                "xi": np.ascontiguousarray(x_imag[:, sl]).reshape(NB, -1),
                "grr": np.ascontiguousarray(gamma_rr[sl]).reshape(-1),
                "gri": np.ascontiguousarray(gamma_ri[sl]).reshape(-1),
                "gii": np.ascontiguousarray(gamma_ii[sl]).reshape(-1),
                "bet": np.ascontiguousarray(beta[sl]).reshape(-1),
            }
        )
    return in_maps


def assemble_output(results) -> np.ndarray:
    """Gather per-core fp16 (re, im) outputs into the full complex64 array."""
    out = np.empty((NB, C_FULL, HW), dtype=np.complex64)
    for c in range(N_CORES):
        o_r = np.asarray(results[c]["outr"])  # [NB, NPOS] fp16
        o_i = np.asarray(results[c]["outi"])
        sl = slice(c * C_LOC, (c + 1) * C_LOC)
        out.real[:, sl] = o_r.astype(np.float32).reshape(NB, C_LOC, HW)
        out.imag[:, sl] = o_i.astype(np.float32).reshape(NB, C_LOC, HW)
    return out.reshape(NB, C_FULL, 256, 256)


def kernel(x_real, x_imag, gamma_rr, gamma_ri, gamma_ii, beta) -> np.ndarray:
    x_real = np.asarray(x_real, dtype=np.float32)
    x_imag = np.asarray(x_imag, dtype=np.float32)
    gamma_rr = np.asarray(gamma_rr, dtype=np.float32)
    gamma_ri = np.asarray(gamma_ri, dtype=np.float32)
    gamma_ii = np.asarray(gamma_ii, dtype=np.float32)
    beta = np.asarray(beta, dtype=np.float32)

    nc = _get_nc(NPOS_FULL)
    in_maps = make_in_maps(x_real, x_imag, gamma_rr, gamma_ri, gamma_ii, beta)
    res = run_bass_kernel_spmd(nc, in_maps, core_ids=list(range(N_CORES)))
    return assemble_output(res.results)


# revision 13
# speedup vs baseline: 1.3641x; 1.1264x over previous
"""Complex batch-norm Trainium2 kernel (nn_ComplexBatchNormal).

Full inputs: x_real/x_imag [16, 32, 256, 256] f32, params [32, 256, 256] f32.
Output: complex64 [16, 32, 256, 256].

Sharding: channels C=32 split across 8 cores (4 channels each) -> fully local
batch statistics per core, no collectives.

Per-core algorithm (positions N = 4*256*256 = 262144, batch B = 16, 4 tiles
of [128, 512] positions):
  pass 1: load x f32, cast to fp16 (ACT for xr, DVE for xi), squares on ACT,
          product on DVE; 5 stats accumulated over B via TensorE identity
          matmuls into 5 PSUM banks.
  coef:   per tile-PAIR (FD=1024 ops), fp16 throughout: analytic inverse-sqrt
          of the 2x2 covariance folded with gamma/beta/mu into 6 fp16
          coefficients a1,a2,a0,b1,b2,b0.
  pass 2: DVE muls U=a1*xr, V=a2*xi (fp16 2x, quarter-batch granularity);
          U+V+bias summed on TensorE into PSUM (3 streams/comp/sample);
          per-sample PSUM->SBUF fp16 copyouts split DVE/ACT; outputs are two
          fp16 DRAM tensors (re, im) upcast + combined on host.
"""

import sys

if "/opt/trn_rl_repo" not in sys.path:
    sys.path.insert(0, "/opt/trn_rl_repo")

from contextlib import ExitStack

import numpy as np

import concourse.bacc as bacc
import concourse.bass as bass
import concourse.tile as tile
from concourse import masks, mybir
from concourse.bass_utils import run_bass_kernel_spmd

P = 128          # SBUF partitions
F = 512          # free-dim positions per tile (= one PSUM bank of f32)
NB = 16          # batch size
G = 4            # batch-samples per load group
QB = 2           # batch-samples per pass-2 mul/out group
EPS = 1e-5
N_CORES = 8
C_FULL = 32
C_LOC = C_FULL // N_CORES  # 4 channels per core
HW = 256 * 256
NPOS_FULL = C_LOC * HW     # 262144 positions per core

f32 = mybir.dt.float32
f16 = mybir.dt.float16

ACT = mybir.ActivationFunctionType


def bcast_free(ap: bass.AP, n: int) -> bass.AP:
    """View [P, F] as [P, n, F] with the middle dim broadcast (step 0)."""
    return bass.AP(tensor=ap.tensor, offset=ap.offset, ap=[ap.ap[0], [0, n], ap.ap[1]])


def _emit(nc: bacc.Bacc, ctx: ExitStack, tc: "tile.TileContext", npos: int,
          dve_copyout_share: int = 3):
    NT = npos // (P * F)
    assert NT * P * F == npos and NT % 2 == 0

    xr_d = nc.dram_tensor("xr", [NB, npos], f32, kind="ExternalInput")
    xi_d = nc.dram_tensor("xi", [NB, npos], f32, kind="ExternalInput")
    grr_d = nc.dram_tensor("grr", [npos], f32, kind="ExternalInput")
    gri_d = nc.dram_tensor("gri", [npos], f32, kind="ExternalInput")
    gii_d = nc.dram_tensor("gii", [npos], f32, kind="ExternalInput")
    bet_d = nc.dram_tensor("bet", [npos], f32, kind="ExternalInput")
    or_d = nc.dram_tensor("outr", [NB, npos], f16, kind="ExternalOutput")
    oi_d = nc.dram_tensor("outi", [NB, npos], f16, kind="ExternalOutput")

    xr_gv = xr_d.ap().rearrange("(g q) (t p f) -> g t p q f", q=G, p=P, f=F)
    xi_gv = xi_d.ap().rearrange("(g q) (t p f) -> g t p q f", q=G, p=P, f=F)
    grr_v = grr_d.ap().rearrange("(t p f) -> t p f", p=P, f=F)
    gri_v = gri_d.ap().rearrange("(t p f) -> t p f", p=P, f=F)
    gii_v = gii_d.ap().rearrange("(t p f) -> t p f", p=P, f=F)
    bet_v = bet_d.ap().rearrange("(t p f) -> t p f", p=P, f=F)
    or_v = or_d.ap().rearrange("(q b) (t p f) -> q t p b f", b=QB, p=P, f=F)
    oi_v = oi_d.ap().rearrange("(q b) (t p f) -> q t p b f", b=QB, p=P, f=F)

    singles = ctx.enter_context(tc.tile_pool(name="singles", bufs=1))
    xpool = ctx.enter_context(tc.tile_pool(name="x", bufs=2))      # f32 staging
    xbpool = ctx.enter_context(tc.tile_pool(name="xb", bufs=2))    # fp16 resident
    sqpool = ctx.enter_context(tc.tile_pool(name="sq", bufs=1))
    gpool = ctx.enter_context(tc.tile_pool(name="g", bufs=1))
    epool = ctx.enter_context(tc.tile_pool(name="ex", bufs=1))     # stat extracts (pairs)
    cpool = ctx.enter_context(tc.tile_pool(name="coef", bufs=2))   # coef scratch (pairs)
    wpool = ctx.enter_context(tc.tile_pool(name="w", bufs=1))      # U/V tiles
    opool = ctx.enter_context(tc.tile_pool(name="o", bufs=2))      # fp16 out tiles
    psum = ctx.enter_context(tc.tile_pool(name="ps", bufs=1, space="PSUM"))

    ident = singles.tile([P, P], f32)
    masks.make_identity(nc, ident[:])
    identb = singles.tile([P, P], f16)
    nc.scalar.copy(identb[:], ident[:])

    inv16 = 1.0 / NB

    NPAIR = NT // 2

    # per-pair persistent handles
    ex_tiles = {}
    coef_tiles = {}

    prefetched = {}

    def prefetch_loads(t, ngroups=2):
        """Issue the first x-load DMAs of tile t early (fills DMA idle during
        the coefficient phase; limited by xg buffer rotation depth)."""
        for g in range(ngroups):
            xg = xpool.tile([P, G, F], f32, tag="xr", name=f"xr{t}_{g}")
            nc.sync.dma_start(xg[:], xr_gv[g, t])
            yg = xpool.tile([P, G, F], f32, tag="xi", name=f"xi{t}_{g}")
            nc.sync.dma_start(yg[:], xi_gv[g, t])
            prefetched[(t, g)] = (xg, yg)

    def pass1(t):
        """Load + cast + squares + product + stat matmuls for tile t."""
        XB = xbpool.tile([P, NB, F], f16, tag="XB", name=f"XB{t}")
        XIB = xbpool.tile([P, NB, F], f16, tag="XIB", name=f"XIB{t}")
        for g in range(NB // G):
            if (t, g) in prefetched:
                xg, yg = prefetched.pop((t, g))
            else:
                xg = xpool.tile([P, G, F], f32, tag="xr", name=f"xr{t}_{g}")
                nc.sync.dma_start(xg[:], xr_gv[g, t])
                yg = xpool.tile([P, G, F], f32, tag="xi", name=f"xi{t}_{g}")
                nc.sync.dma_start(yg[:], xi_gv[g, t])

            XBg = XB[:, g * G:(g + 1) * G, :]
            XIBg = XIB[:, g * G:(g + 1) * G, :]
            nc.scalar.copy(XBg, xg[:])            # ACT cast
            nc.vector.tensor_copy(XIBg, yg[:])    # DVE cast (2x-2p)

            sq_r = sqpool.tile([P, G, F], f16, tag="sqr", name=f"sqr{t}_{g}")
            sq_i = sqpool.tile([P, G, F], f16, tag="sqi", name=f"sqi{t}_{g}")
            p_g = sqpool.tile([P, G, F], f16, tag="pg", name=f"pg{t}_{g}")
            if t < 2:
                # ramp phase: DVE is idle while DMA/ACT pace pass 1 -> square
                # on DVE from the fp16 tiles (2x mode)
                nc.vector.tensor_mul(sq_r[:], XBg, XBg)
                nc.vector.tensor_mul(sq_i[:], XIBg, XIBg)
            else:
                nc.scalar.square(sq_r[:], xg[:])      # ACT
                nc.scalar.square(sq_i[:], yg[:])      # ACT
            nc.vector.tensor_mul(p_g[:], XBg, XIBg)  # DVE 2x

            for j in range(G):
                b = g * G + j
                st = b == 0
                sp = b == NB - 1
                nc.tensor.matmul(S_r[:], identb[:], XB[:, b, :], start=st, stop=sp)
                nc.tensor.matmul(S_i[:], identb[:], XIB[:, b, :], start=st, stop=sp)
                nc.tensor.matmul(S_rr[:], identb[:], sq_r[:, j, :], start=st, stop=sp)
                nc.tensor.matmul(S_ii[:], identb[:], sq_i[:, j, :], start=st, stop=sp)
                nc.tensor.matmul(S_ri[:], identb[:], p_g[:, j, :], start=st, stop=sp)
        return XB, XIB

    def extracts(t):
        """ACT: PSUM stat sums -> fp16 mean/var tiles (per tile)."""
        e = {
            nm: epool.tile([P, F], f16, tag=nm, name=f"{nm}_{t}", bufs=2)
            for nm in ("mu_r", "mu_i", "Vrr", "Vii", "Vri")
        }
        ex_tiles[t] = e
        nc.scalar.activation(e["mu_r"][:], S_r[:], ACT.Copy, scale=inv16)
        nc.scalar.activation(e["mu_i"][:], S_i[:], ACT.Copy, scale=inv16)
        nc.scalar.activation(e["Vrr"][:], S_rr[:], ACT.Copy, bias=EPS, scale=inv16)
        nc.scalar.activation(e["Vii"][:], S_ii[:], ACT.Copy, bias=EPS, scale=inv16)
        nc.scalar.activation(e["Vri"][:], S_ri[:], ACT.Copy, scale=inv16)

    par_tiles = {}

    def load_params(t):
        """Prefetch gamma/beta f32 for tile t."""
        gr = gpool.tile([P, F], f32, tag="grr", name=f"grr{t}", bufs=2)
        gi = gpool.tile([P, F], f32, tag="gri", name=f"gri{t}", bufs=2)
        gg = gpool.tile([P, F], f32, tag="gii", name=f"gii{t}", bufs=2)
        bt = gpool.tile([P, F], f32, tag="bet", name=f"bet{t}", bufs=2)
        nc.sync.dma_start(gr[:], grr_v[t])
        nc.sync.dma_start(gi[:], gri_v[t])
        nc.sync.dma_start(gg[:], gii_v[t])
        nc.sync.dma_start(bt[:], bet_v[t])
        par_tiles[t] = (gr, gi, gg, bt)

    def coef(k):
        """Per-tile coefficient math, fp16, FD=512 ops."""
        e = ex_tiles[k]
        mu_r, mu_i = e["mu_r"], e["mu_i"]
        Vrr, Vii, Vri = e["Vrr"], e["Vii"], e["Vri"]
        gr, gi, gg, bt = par_tiles[k]

        cp = lambda tag: cpool.tile([P, F], f16, tag=tag, name=f"{tag}{k}", bufs=2)
        gr16 = cp("gr16"); nc.vector.tensor_copy(gr16[:], gr[:])
        gi16 = cp("gi16"); nc.vector.tensor_copy(gi16[:], gi[:])
        gg16 = cp("gg16"); nc.vector.tensor_copy(gg16[:], gg[:])
        bt16 = cp("bt16"); nc.vector.tensor_copy(bt16[:], bt[:])

        mr2 = cp("s0")
        nc.scalar.square(mr2[:], mu_r[:])
        nc.vector.tensor_sub(Vrr[:], Vrr[:], mr2[:])
        mi2 = cp("s1")
        nc.scalar.square(mi2[:], mu_i[:])
        nc.vector.tensor_sub(Vii[:], Vii[:], mi2[:])
        mri = cp("s2")
        nc.vector.tensor_mul(mri[:], mu_r[:], mu_i[:])
        nc.vector.tensor_sub(Vri[:], Vri[:], mri[:])

        tau = cp("s3")
        nc.vector.tensor_add(tau[:], Vrr[:], Vii[:])
        det = cp("s4")
        nc.vector.tensor_mul(det[:], Vrr[:], Vii[:])
        vri2 = cp("s0")
        nc.scalar.square(vri2[:], Vri[:])
        nc.vector.tensor_sub(det[:], det[:], vri2[:])

        s_s = cp("s5")
        nc.scalar.sqrt(s_s[:], det[:])
        nc.vector.scalar_tensor_tensor(
            tau[:], s_s[:], 2.0, tau[:], mybir.AluOpType.mult, mybir.AluOpType.add
        )
        t_t = cp("s6")
        nc.scalar.sqrt(t_t[:], tau[:])
        inv = cp("s8")
        st32 = cpool.tile([P, F], f32, tag="st32", name=f"st32_{k}", bufs=1)
        nc.vector.tensor_mul(st32[:], s_s[:], t_t[:])
        inv32 = cpool.tile([P, F], f32, tag="inv32", name=f"inv32_{k}", bufs=1)
        nc.vector.reciprocal_approx_fast(inv32[:], st32[:])
        nc.vector.tensor_copy(inv[:], inv32[:])

        # W matrix (Wri holds +Vri*inv; signs folded below)
        nc.vector.tensor_add(Vii[:], Vii[:], s_s[:])
        Wrr = cp("s3")
        nc.vector.tensor_mul(Wrr[:], Vii[:], inv[:])
        nc.vector.tensor_add(Vrr[:], Vrr[:], s_s[:])
        Wii = cp("s4")
        nc.vector.tensor_mul(Wii[:], Vrr[:], inv[:])
        Wri = cp("s5")
        nc.vector.tensor_mul(Wri[:], Vri[:], inv[:])

        co = {nm: cpool.tile([P, F], f16, tag=nm, name=f"{nm}_{k}")
              for nm in ("a1", "a2", "b1", "b2", "a0", "b0")}
        m2 = cp("s6")
        nc.vector.tensor_mul(co["a1"][:], gr16[:], Wrr[:])
        nc.vector.tensor_mul(m2[:], gi16[:], Wri[:])
        nc.vector.tensor_sub(co["a1"][:], co["a1"][:], m2[:])

        m4 = cp("s7")
        nc.vector.tensor_mul(co["a2"][:], gi16[:], Wii[:])
        nc.vector.tensor_mul(m4[:], gr16[:], Wri[:])
        nc.vector.tensor_sub(co["a2"][:], co["a2"][:], m4[:])

        m6 = cp("s8")
        nc.vector.tensor_mul(co["b1"][:], gi16[:], Wrr[:])
        nc.vector.tensor_mul(m6[:], gg16[:], Wri[:])
        nc.vector.tensor_sub(co["b1"][:], co["b1"][:], m6[:])

        nc.vector.tensor_mul(co["b2"][:], gg16[:], Wii[:])
        nc.vector.tensor_sub(co["b2"][:], co["b2"][:], m2[:])

        n1 = cp("s0")
        nc.vector.tensor_mul(n1[:], co["a1"][:], mu_r[:])
        nc.vector.tensor_sub(co["a0"][:], bt16[:], n1[:])
        n2 = cp("s1")
        nc.vector.tensor_mul(n2[:], co["a2"][:], mu_i[:])
        nc.vector.tensor_sub(co["a0"][:], co["a0"][:], n2[:])

        n3 = cp("s2")
        nc.vector.tensor_mul(n3[:], co["b1"][:], mu_r[:])
        nc.vector.tensor_sub(co["b0"][:], bt16[:], n3[:])
        n4 = cp("s6")
        nc.vector.tensor_mul(n4[:], co["b2"][:], mu_i[:])
        nc.vector.tensor_sub(co["b0"][:], co["b0"][:], n4[:])

        coef_tiles[k] = co

    nfin = [0]

    def pass2(t, XB, XIB):
        """DVE muls + TensorE sum streams + split copyouts + DMA out."""
        co = coef_tiles[t]
        a1 = co["a1"][:]
        a2 = co["a2"][:]
        b1 = co["b1"][:]
        b2 = co["b2"][:]
        a0 = co["a0"][:]
        b0 = co["b0"][:]
        for q in range(NB // QB):
            b0_ = q * QB
            XBq = XB[:, b0_:b0_ + QB, :]
            XIBq = XIB[:, b0_:b0_ + QB, :]
            U = wpool.tile([P, QB, F], f16, tag="U", name=f"U{t}_{q}")
            nc.vector.tensor_mul(U[:], XBq, bcast_free(a1, QB))
            V = wpool.tile([P, QB, F], f16, tag="V", name=f"V{t}_{q}")
            nc.vector.tensor_mul(V[:], XIBq, bcast_free(a2, QB))
            U2 = wpool.tile([P, QB, F], f16, tag="U2", name=f"U2{t}_{q}")
            nc.vector.tensor_mul(U2[:], XBq, bcast_free(b1, QB))
            V2 = wpool.tile([P, QB, F], f16, tag="V2", name=f"V2{t}_{q}")
            nc.vector.tensor_mul(V2[:], XIBq, bcast_free(b2, QB))

            o_r = opool.tile([P, QB, F], f16, tag="or", name=f"or{t}_{q}")
            o_i = opool.tile([P, QB, F], f16, tag="oi", name=f"oi{t}_{q}")
            for j in range(QB):
                for comp, (Uc, Vc, cc, oc) in enumerate(
                    ((U, V, a0, o_r), (U2, V2, b0, o_i))
                ):
                    PS = psum.tile([P, F], f32, tag="PS",
                                   name=f"PS{t}_{q}_{j}_{comp}", bufs=3)
                    nc.tensor.matmul(PS[:], identb[:], Uc[:, j, :], start=True, stop=False)
                    nc.tensor.matmul(PS[:], identb[:], Vc[:, j, :], start=False, stop=False)
                    nc.tensor.matmul(PS[:], identb[:], cc, start=False, stop=True)
                    share = 0 if t == NT - 1 else dve_copyout_share
                    if nfin[0] % 16 < share:
                        nc.vector.tensor_copy(oc[:, j, :], PS[:])
                    else:
                        nc.scalar.copy(oc[:, j, :], PS[:])
                    nfin[0] += 1
            nc.sync.dma_start(or_v[q, t], o_r[:])
            nc.sync.dma_start(oi_v[q, t], o_i[:])

    # ---- main schedule: pairs of tiles, software-pipelined ----
    global S_r, S_i, S_rr, S_ii, S_ri
    S_r = psum.tile([P, F], f32, tag="S_r")
    S_i = psum.tile([P, F], f32, tag="S_i")
    S_rr = psum.tile([P, F], f32, tag="S_rr")
    S_ii = psum.tile([P, F], f32, tag="S_ii")
    S_ri = psum.tile([P, F], f32, tag="S_ri")

    load_params(0)
    for t in range(NT):
        xb = pass1(t)
        extracts(t)
        if t + 1 < NT:
            load_params(t + 1)
            prefetch_loads(t + 1, ngroups=2)
        coef(t)
        pass2(t, *xb)


def build_nc(npos: int = NPOS_FULL, dve_copyout_share: int = 3) -> bacc.Bacc:
    nc = bacc.Bacc("TRN2", target_bir_lowering=False, debug=False)
    with tile.TileContext(nc) as tc:
        with ExitStack() as ctx:
            _emit(nc, ctx, tc, npos, dve_copyout_share=dve_copyout_share)
    nc.compile()
    return nc


_cache: dict = {}


def _get_nc(npos: int = NPOS_FULL, dve_copyout_share: int = 3) -> bacc.Bacc:
    key = (npos, dve_copyout_share)
    if key not in _cache:
        _cache[key] = build_nc(npos, dve_copyout_share)
    return _cache[key]


def make_in_maps(x_real, x_imag, gamma_rr, gamma_ri, gamma_ii, beta):
    """Shard channels across cores; returns per-core input dicts."""
    in_maps = []
    for c in range(N_CORES):
        sl = slice(c * C_LOC, (c + 1) * C_LOC)
        in_maps.append(
            {
                "xr": np.ascontiguousarray(x_real[:, sl]).reshape(NB, -1),
                "xi": np.ascontiguousarray(x_imag[:, sl]).reshape(NB, -1),
                "grr": np.ascontiguousarray(gamma_rr[sl]).reshape(-1),
                "gri": np.ascontiguousarray(gamma_ri[sl]).reshape(-1),
                "gii": np.ascontiguousarray(gamma_ii[sl]).reshape(-1),
                "bet": np.ascontiguousarray(beta[sl]).reshape(-1),
            }
        )
    return in_maps


def assemble_output(results) -> np.ndarray:
    """Gather per-core fp16 (re, im) outputs into the full complex64 array."""
    out = np.empty((NB, C_FULL, HW), dtype=np.complex64)
    for c in range(N_CORES):
        o_r = np.asarray(results[c]["outr"])  # [NB, NPOS] fp16
        o_i = np.asarray(results[c]["outi"])
        sl = slice(c * C_LOC, (c + 1) * C_LOC)
        out.real[:, sl] = o_r.astype(np.float32).reshape(NB, C_LOC, HW)
        out.imag[:, sl] = o_i.astype(np.float32).reshape(NB, C_LOC, HW)
    return out.reshape(NB, C_FULL, 256, 256)


def kernel(x_real, x_imag, gamma_rr, gamma_ri, gamma_ii, beta) -> np.ndarray:
    x_real = np.asarray(x_real, dtype=np.float32)
    x_imag = np.asarray(x_imag, dtype=np.float32)
    gamma_rr = np.asarray(gamma_rr, dtype=np.float32)
    gamma_ri = np.asarray(gamma_ri, dtype=np.float32)
    gamma_ii = np.asarray(gamma_ii, dtype=np.float32)
    beta = np.asarray(beta, dtype=np.float32)

    nc = _get_nc(NPOS_FULL)
    in_maps = make_in_maps(x_real, x_imag, gamma_rr, gamma_ri, gamma_ii, beta)
    res = run_bass_kernel_spmd(nc, in_maps, core_ids=list(range(N_CORES)))
    return assemble_output(res.results)
